# revision 16
# baseline (speedup 1.0000x reference)
"""CTPN loss kernel for 8 Trainium2 NeuronCores.

Strategy (data parallel over anchors, maps sharded by position):
  * The H*W=24576 spatial positions are split into 8 contiguous slices of
    3072; core c holds the dense map data for its slice, re-laid-out into an
    SBUF-friendly [128, 1536] f32 tile of "channel-half" rows.
  * All index lists (positive/negative/vertical/side) are bucketed on the
    host by position -> core, and inside a core by (channel, pos-half) ->
    16-partition GPSIMD group.  One InstIndirectCopy per core gathers every
    referenced value (the gather is the whole memory-bound core of this
    loss).
  * Smooth-L1 is evaluated with the identity
        sl1(d) = 0.5*m^2 + |d| - m,   m = min(|d|, 1)
    so only three masked free-dim reductions are needed; per-partition
    partial sums go back to the host, which applies the per-segment
    divisors (1/(2*Nv), 1/No, 1/Ns) and sums across cores (the all-reduce).
  * Classification CE uses ce_pos = softplus(l0-l1), ce_neg = softplus(l1-l0)
    on pair-adjacent gathered columns.
"""

import sys

sys.path.insert(0, "/opt/trn_rl_repo")

import numpy as np

import concourse.bacc as bacc
import concourse.tile as tile
from concourse import mybir
from concourse import bass_utils

# ---------------- problem constants (hardcoded per contract) ----------------
H, W, K = 128, 192, 10
HW = H * W                     # 24576
N_CORES = 8
PPC = HW // N_CORES            # 3072 positions per core
COLS = 1536                    # slot width (elements) = half of PPC
QCOLS = 768                    # quarter width (score slots are pair-interleaved)
NS = 128.0
NV_REG = 20000
NO_REG = 5000

# ---- static unit tables ----------------------------------------------------
# unit kinds: 'vp' (a, h) -> 2 partitions; 'sd' (a, h) -> 1; 'sc' (a, q) -> 1
UNITS = []
for a in range(K):
    for h in range(2):
        UNITS.append(("vp", a, h))
for a in range(K):
    for h in range(2):
        UNITS.append(("sd", a, h))
for a in range(K):
    for q in range(4):
        UNITS.append(("sc", a, q))
N_UNITS = len(UNITS)  # 80
UNIT_NPART = {"vp": 2, "sd": 1, "sc": 1}

_cache = {}


def _pack_units(main_cnt, cls_cnt):
    """Greedy LPT bin-pack of units into 8 groups of <=16 partitions.

    main_cnt/cls_cnt: [N_UNITS] entry counts for one core.
    Returns: group id per unit, per-group (n_main, n_cls).
    """
    order = np.argsort(-(main_cnt + cls_cnt), kind="stable")
    gmain = [0] * 8
    gcls = [0] * 8
    gpart = [0] * 8
    ugroup = [0] * N_UNITS
    for ui in order:
        npart = UNIT_NPART[UNITS[ui][0]]
        best, bestv = -1, None
        for g in range(8):
            if gpart[g] + npart > 16:
                continue
            v = gmain[g] + gcls[g]
            if bestv is None or v < bestv:
                best, bestv = g, v
        assert best >= 0, "unit packing overflow"
        ugroup[ui] = best
        gmain[best] += int(main_cnt[ui])
        gcls[best] += int(cls_cnt[ui])
        gpart[best] += npart
    return ugroup, gmain, gcls


def _build_bass(NV, C0, WB, NCLS):
    nc = bacc.Bacc("TRN2", target_bir_lowering=False)
    NI = NV // 16
    MEGA = nc.dram_tensor("mega", [128, WB], mybir.dt.uint8, kind="ExternalInput")
    OUT = nc.dram_tensor("out", [128, 4], mybir.dt.float32, kind="ExternalOutput")

    o_data = 0
    o_tm = 6144
    o_idx = o_tm + 8 * NV
    o_mc = o_idx + 2 * NI

    f32 = mybir.dt.float32
    with tile.TileContext(nc) as tc:
        with tc.tile_pool(name="p", bufs=1) as pool:
            mega = pool.tile([128, WB], mybir.dt.uint8)
            # phase A: data + idx (what the gather needs)
            nc.sync.dma_start(mega[:, o_data:6144], MEGA[:, o_data:6144])
            nc.sync.dma_start(mega[:, o_idx:o_mc], MEGA[:, o_idx:o_mc])
            # phase B: targets + cls mask (needed only after the gather)
            nc.sync.dma_start(mega[:, o_tm:o_idx], MEGA[:, o_tm:o_idx])
            nc.sync.dma_start(mega[:, o_mc:WB], MEGA[:, o_mc:WB])

            # hoist both activation-table loads off the critical path: these
            # dummy ops touch every func class we use before the gather runs
            warm = pool.tile([128, 4], f32)
            nc.scalar.activation(warm[:, 0:2], warm[:, 2:4],
                                 mybir.ActivationFunctionType.Ln)
            nc.scalar.activation(warm[:, 0:2], warm[:, 2:4],
                                 mybir.ActivationFunctionType.Exp)
            nc.scalar.activation(warm[:, 0:2], warm[:, 2:4],
                                 mybir.ActivationFunctionType.Abs)
            nc.scalar.activation(warm[:, 0:2], warm[:, 2:4],
                                 mybir.ActivationFunctionType.Square)

            data_v = mega[:, o_data:6144].bitcast(f32)           # [128,1536]
            idx_v = mega[:, o_idx:o_idx + 2 * NI].bitcast(mybir.dt.uint16)
            tm_v = mega[:, o_tm:o_tm + 8 * NV].bitcast(f32)      # [128,2NV]
            mcls_v = mega[:, o_mc:o_mc + NCLS]                   # u8 [128,NCLS]

            g = pool.tile([128, NV], f32)
            nc.gpsimd.indirect_copy(
                g[:], data_v, idx_v, i_know_ap_gather_is_preferred=True
            )

            # dm[p, r, k] = g[p, k] - TM[p, r, k]; TM defaults to the value
            # the gather produces, so non-anchor slots give exactly 0
            gb = g[:, None, :].to_broadcast([128, 2, NV])
            dm = pool.tile([128, 2 * NV], f32)
            nc.vector.tensor_tensor(dm[:].rearrange("p (r k) -> p r k", r=2),
                                    gb,
                                    tm_v.rearrange("p (r k) -> p r k", r=2),
                                    op=mybir.AluOpType.subtract)

            P = pool.tile([128, 4], f32)
            # A = |dm| on the scalar engine (Abs is in every act table);
            # its accum_out gives P[:,0] = sum(|dm|) for free
            A = pool.tile([128, 2 * NV], f32)
            nc.scalar.activation(A[:], dm[:],
                                 mybir.ActivationFunctionType.Abs,
                                 accum_out=P[:, 0:1])
            # m = min(|dm|, 1)
            m = pool.tile([128, 2 * NV], f32)
            nc.vector.tensor_scalar(m[:], A[:], 1.0, None,
                                    mybir.AluOpType.min)
            # P[:,1] = sum(m)
            nc.vector.tensor_reduce(P[:, 1:2], m[:],
                                    axis=mybir.AxisListType.X,
                                    op=mybir.AluOpType.add)
            # P[:,2] = sum(m*m) via ACT Square with accumulate
            sq = pool.tile([128, 2 * NV], f32)
            nc.scalar.activation(sq[:], m[:],
                                 mybir.ActivationFunctionType.Square,
                                 accum_out=P[:, 2:3])

            # classification tail: columns [C0, NV) hold 2*NCLS gathered
            # logits, pair-adjacent; ce = softplus(first - second)
            dc = pool.tile([128, NCLS], f32)
            nc.vector.tensor_tensor(dc[:], g[:, C0:NV:2], g[:, C0 + 1:NV:2],
                                    op=mybir.AluOpType.subtract)
            # ce = softplus(d) = ln(exp(d) + 1); Exp and Ln share one
            # activation table (natural_log_exp_and_others)
            ex = pool.tile([128, NCLS], f32)
            nc.scalar.activation(ex[:], dc[:],
                                 mybir.ActivationFunctionType.Exp)
            ce = pool.tile([128, NCLS], f32)
            nc.scalar.activation(ce[:], ex[:],
                                 mybir.ActivationFunctionType.Ln, bias=1.0)
            cj = pool.tile([128, NCLS], f32)
            nc.vector.tensor_tensor(cj[:], ce[:], mcls_v,
                                    op=mybir.AluOpType.mult)
            nc.vector.tensor_reduce(P[:, 3:4], cj[:],
                                    axis=mybir.AxisListType.X,
                                    op=mybir.AluOpType.add)

            nc.sync.dma_start(OUT[:, :], P[:])
    nc.compile()
    return nc


def kernel(**inputs):
    score = np.asarray(inputs["score"], dtype=np.float32)[0]            # [20,H,W]
    vp = np.asarray(inputs["vertical_pred"], dtype=np.float32)[0]
    side = np.asarray(inputs["side_refinement"], dtype=np.float32)[0]   # [10,H,W]
    pidx = np.asarray(inputs["positive"])
    nidx = np.asarray(inputs["negative"])
    vidx = np.asarray(inputs["vertical_reg_idx"])
    vtgt = np.asarray(inputs["vertical_reg_tgt"], dtype=np.float32)
    sidx = np.asarray(inputs["side_reg_idx"])
    stgt = np.asarray(inputs["side_reg_tgt"], dtype=np.float32)

    score_f = score.reshape(2 * K, HW)
    vp_f = vp.reshape(2 * K, HW)
    side_f = side.reshape(K, HW)

    def fields(idx):
        x = idx[:, 0].astype(np.int64)
        y = idx[:, 1].astype(np.int64)
        a = idx[:, 2].astype(np.int64)
        pos = y * W + x
        return a, pos // PPC, pos % PPC

    va, vcore, vposl = fields(vidx)
    sa, score_, sposl = fields(sidx)
    pa, pcore, pposl = fields(pidx)
    na, ncore, nposl = fields(nidx)

    # --- per (core, unit) entry lists -------------------------------------
    # main entries: vp + sd; cls entries: sc (two idx slots per anchor)
    v_h = vposl // COLS
    v_u = (vposl % COLS).astype(np.int64)
    v_unit = (va * 2 + v_h).astype(np.int64)                 # vp units 0..19
    s_h = sposl // COLS
    s_u = (sposl % COLS).astype(np.int64)
    s_unit = (20 + sa * 2 + s_h).astype(np.int64)            # sd units 20..39
    p_q = pposl // QCOLS
    p_u = (2 * (pposl % QCOLS)).astype(np.int64)
    p_unit = (40 + pa * 4 + p_q).astype(np.int64)            # sc units 40..79
    n_q = nposl // QCOLS
    n_u = (2 * (nposl % QCOLS)).astype(np.int64)
    n_unit = (40 + na * 4 + n_q).astype(np.int64)

    main_core = np.concatenate([vcore, score_])
    main_unit = np.concatenate([v_unit, s_unit])
    main_u = np.concatenate([v_u, s_u])
    main_t0 = np.concatenate([vtgt[:, 0], stgt])
    main_t1 = np.concatenate([vtgt[:, 1], np.zeros_like(stgt)])
    main_isv = np.concatenate(
        [np.ones(len(vidx), np.bool_), np.zeros(len(sidx), np.bool_)])

    cls_core = np.concatenate([pcore, ncore])
    cls_unit = np.concatenate([p_unit, n_unit])
    cls_u = np.concatenate([p_u, n_u])
    cls_ispos = np.concatenate(
        [np.ones(len(pidx), np.bool_), np.zeros(len(nidx), np.bool_)])

    main_cnt = np.zeros((N_CORES, N_UNITS), np.int64)
    np.add.at(main_cnt, (main_core, main_unit), 1)
    cls_cnt = np.zeros((N_CORES, N_UNITS), np.int64)
    np.add.at(cls_cnt, (cls_core, cls_unit), 2)

    # --- pack units into groups per core ----------------------------------
    packs = [_pack_units(main_cnt[c], cls_cnt[c]) for c in range(N_CORES)]
    c0 = max(max(p[1]) for p in packs)
    c0 += c0 % 2
    max_cls = max(max(p[2]) for p in packs)
    NV = c0 + max_cls
    NV = ((NV + 15) // 16) * 16
    NCLS = (NV - c0) // 2
    NI = NV // 16
    WB = 6144 + 8 * NV + 2 * NI + NCLS
    WB = ((WB + 3) // 4) * 4

    key = (NV, c0)
    if key not in _cache:
        _cache[key] = _build_bass(NV, c0, WB, NCLS)
    nc = _cache[key]

    o_tm = 6144
    o_idx = o_tm + 8 * NV
    o_mc = o_idx + 2 * NI

    in_maps = []
    wvec_v = np.zeros((N_CORES, 128), np.float32)
    wvec_o = np.zeros((N_CORES, 128), np.float32)
    for c in range(N_CORES):
        ugroup, gmain, gcls = packs[c]
        # partition layout: group g owns partitions 16g..16g+15, assigned in
        # unit-pack order
        gnext = [16 * g for g in range(8)]
        upart = [0] * N_UNITS
        for ui in range(N_UNITS):
            g = ugroup[ui]
            upart[ui] = gnext[g]
            gnext[g] += UNIT_NPART[UNITS[ui][0]]
            assert gnext[g] <= 16 * g + 16

        data = np.zeros((128, COLS), np.float32)
        base = c * PPC
        for ui, (kind, a, hq) in enumerate(UNITS):
            p0 = upart[ui]
            if kind == "vp":
                sl = slice(base + hq * COLS, base + (hq + 1) * COLS)
                data[p0] = vp_f[2 * a, sl]
                data[p0 + 1] = vp_f[2 * a + 1, sl]
                wvec_v[c, p0] = wvec_v[c, p0 + 1] = 1.0 / (2.0 * NV_REG)
            elif kind == "sd":
                sl = slice(base + hq * COLS, base + (hq + 1) * COLS)
                data[p0] = side_f[a, sl]
                wvec_o[c, p0] = 1.0 / NO_REG
            else:  # sc, pair-interleaved quarter
                sl = slice(base + hq * QCOLS, base + (hq + 1) * QCOLS)
                data[p0, 0::2] = score_f[2 * a, sl]
                data[p0, 1::2] = score_f[2 * a + 1, sl]

        idxs = np.zeros((128, NI), np.uint16)
        ucol = np.zeros((8, NV), np.int64)    # per-group gathered column
        mcls = np.zeros((128, NCLS), np.uint8)

        gq_main = [0] * 8   # next main col per group
        gq_cls = [0] * 8    # next cls PAIR slot per group

        def put_idx(g, col, val):
            idxs[16 * g + col % 16, col // 16] = val
            ucol[g, col] = val

        # main entries: remember (partition, r, col, target) to overwrite
        ov_p, ov_r, ov_c, ov_t = [], [], [], []
        msel = main_core == c
        for u, ui, t0, t1, isv in zip(main_u[msel], main_unit[msel],
                                      main_t0[msel], main_t1[msel],
                                      main_isv[msel]):
            g = ugroup[ui]
            col = gq_main[g]
            gq_main[g] += 1
            put_idx(g, col, u)
            p0 = upart[ui]
            ov_p.append(p0); ov_r.append(0); ov_c.append(col); ov_t.append(t0)
            if isv:
                ov_p.append(p0 + 1); ov_r.append(1); ov_c.append(col)
                ov_t.append(t1)

        csel = cls_core == c
        for u, ui, ispos in zip(cls_u[csel], cls_unit[csel],
                                cls_ispos[csel]):
            g = ugroup[ui]
            i = gq_cls[g]
            gq_cls[g] += 1
            colf = c0 + 2 * i
            # pos: (l0, l1); neg: (l1, l0) -> ce = softplus(first - second)
            if ispos:
                put_idx(g, colf, u)
                put_idx(g, colf + 1, u + 1)
            else:
                put_idx(g, colf, u + 1)
                put_idx(g, colf + 1, u)
            mcls[upart[ui], i] = 1

        # TM defaults to exactly what the gather will produce (so junk
        # slots subtract to 0), then anchor slots get their real targets
        tm = np.empty((128, 2, NV), np.float32)
        for g in range(8):
            sl = data[16 * g:16 * g + 16][:, ucol[g]]   # [16, NV]
            tm[16 * g:16 * g + 16, 0, :] = sl
            tm[16 * g:16 * g + 16, 1, :] = sl
        if ov_p:
            tm[np.array(ov_p), np.array(ov_r), np.array(ov_c)] = \
                np.array(ov_t, np.float32)

        mega = np.zeros((128, WB), np.uint8)
        mega[:, 0:6144] = data.view(np.uint8).reshape(128, 6144)
        mega[:, o_tm:o_tm + 8 * NV] = tm.view(np.uint8).reshape(128, 8 * NV)
        mega[:, o_idx:o_idx + 2 * NI] = idxs.view(np.uint8).reshape(128, 2 * NI)
        mega[:, o_mc:o_mc + NCLS] = mcls
        in_maps.append({"mega": mega})

    res = bass_utils.run_bass_kernel_spmd(
        nc, in_maps, core_ids=list(range(N_CORES)))

    v_loss = np.float32(0.0)
    o_loss = np.float32(0.0)
    cls_sum = np.float32(0.0)
    for c in range(N_CORES):
        P = res.results[c]["out"]      # [128, 4]
        S = 0.5 * P[:, 2] + P[:, 0] - P[:, 1]
        v_loss += np.float32(np.dot(S, wvec_v[c]))
        o_loss += np.float32(np.dot(S, wvec_o[c]))
        cls_sum += np.float32(P[:, 3].sum())
    cls_loss = np.float32(cls_sum / NS)
    loss = np.float32(cls_loss + v_loss + o_loss)
    return (np.float32(loss), np.float32(cls_loss), np.float32(v_loss),
            np.float32(o_loss))



# revision 19
# speedup vs baseline: 1.5926x; 1.5926x over previous
"""CTPN loss kernel for 8 Trainium2 NeuronCores.

v3 strategy (dense smooth-L1 pass; tiny gather only for dup cells + cls):
  * The H*W=24576 spatial positions are split into 8 contiguous slices of
    3072; core c holds the dense map data for its slice as a bf16
    [128, 1536] tile of "channel-half" rows (score rows pair-interleaved
    by quarter, as the cls gather wants them).  Within each 16-partition
    gather group, vp/sd rows sit at slots {0..3, 8..11} and score rows at
    {4..7, 12..15}, so the target tile T (needed only for vp/sd rows) can
    be DMA'd with a partition-strided pattern that loads every DMA engine
    equally.
  * T holds the per-cell regression targets scattered into the data
    layout, defaulting to a copy of the data so non-anchor cells subtract
    to exactly 0.  Smooth-L1 uses
        sl1(d) = 0.5*d^2 - 0.5*(max(|d|,1)-1)^2
    with Sum(d^2) from a DVE tensor_tensor_reduce and Sum((z-1)^2) from
    one ACT Square (bias=-1) with free accumulation.  z=max(|d|,1) comes
    from bf16 bit tricks (clear sign bit; unsigned max vs 0x3F80).
  * Cells referenced by MORE than one regression entry (random index
    collisions) can hold only one dense target; the 2nd+ entries go
    through a small InstIndirectCopy gather (a few dozen columns), along
    with the 2*128 classification logits (pair-adjacent columns,
    ce = Softplus(first - second), single-table activation).
  * Per-partition partial sums go back to the host, which applies the
    per-segment divisors (1/(2*Nv), 1/No, 1/Ns) and sums across cores
    (the all-reduce of the sharding hint).  Score-row partitions of the
    dense sums are garbage (their T rows are never shipped) and are
    simply not read by the host.
"""

import sys

sys.path.insert(0, "/opt/trn_rl_repo")

import numpy as np

import concourse.bacc as bacc
import concourse.tile as tile
from concourse import mybir
from concourse import bass_utils

# ---------------- problem constants (hardcoded per contract) ----------------
H, W, K = 128, 192, 10
HW = H * W                     # 24576
N_CORES = 8
PPC = HW // N_CORES            # 3072 positions per core
COLS = 1536                    # slot width (elements) = half of PPC
QCOLS = 768                    # quarter width (score slots are pair-interleaved)
NS = 128.0
NV_REG = 20000
NO_REG = 5000

# ---- static unit tables ----------------------------------------------------
# unit kinds: 'vp' (a, h) -> 2 partitions; 'sd' (a, h) -> 1; 'sc' (a, q) -> 1
UNITS = []
for a in range(K):
    for h in range(2):
        UNITS.append(("vp", a, h))
for a in range(K):
    for h in range(2):
        UNITS.append(("sd", a, h))
for a in range(K):
    for q in range(4):
        UNITS.append(("sc", a, q))
N_UNITS = len(UNITS)  # 80
UNIT_NPART = {"vp": 2, "sd": 1, "sc": 1}
REG_UNITS = [ui for ui, u in enumerate(UNITS) if u[0] != "sc"]
SC_UNITS = [ui for ui, u in enumerate(UNITS) if u[0] == "sc"]

_cache = {}


def _bf16(x):
    """f32 ndarray -> uint16 bf16 bits (round to nearest even)."""
    u = np.ascontiguousarray(np.asarray(x, np.float32)).view(np.uint32)
    return (((u + 0x7FFF) + ((u >> 16) & 1)) >> 16).astype(np.uint16)


def _pack_units(main_cnt, cls_cnt):
    """Assign units to 16-partition groups and partition slots.

    vp/sd units go to group-relative slots {0..3, 8..11} (two runs of 4;
    vp needs 2 adjacent slots in one run), sc units to {4..7, 12..15}.
    Balances main_cnt over groups for vp/sd and cls_cnt for sc.
    Returns upart[N_UNITS], ugroup[N_UNITS], per-group (gmain, gcls).
    """
    upart = [-1] * N_UNITS
    ugroup = [-1] * N_UNITS
    gmain = [0] * 8
    gcls = [0] * 8
    # run free-lists per group: [run0_next, run1_next] relative next slot
    reg_runs = [[0, 0] for _ in range(8)]   # used counts in runs of 4
    # vp first (need adjacency), heaviest first
    vps = sorted((ui for ui in REG_UNITS if UNITS[ui][0] == "vp"),
                 key=lambda ui: -main_cnt[ui])
    sds = sorted((ui for ui in REG_UNITS if UNITS[ui][0] == "sd"),
                 key=lambda ui: -main_cnt[ui])
    for ui in vps:
        best, bestv = -1, None
        for g in range(8):
            if max(r for r in (4 - reg_runs[g][0], 4 - reg_runs[g][1])) < 2:
                continue
            if bestv is None or gmain[g] < bestv:
                best, bestv = g, gmain[g]
        assert best >= 0, "vp packing overflow"
        g = best
        r = 0 if 4 - reg_runs[g][0] >= 2 else 1
        base = 16 * g + (0 if r == 0 else 8) + reg_runs[g][r]
        reg_runs[g][r] += 2
        upart[ui] = base
        ugroup[ui] = g
        gmain[g] += int(main_cnt[ui])
    for ui in sds:
        best, bestv = -1, None
        for g in range(8):
            if reg_runs[g][0] >= 4 and reg_runs[g][1] >= 4:
                continue
            if bestv is None or gmain[g] < bestv:
                best, bestv = g, gmain[g]
        assert best >= 0, "sd packing overflow"
        g = best
        r = 0 if reg_runs[g][0] < 4 else 1
        base = 16 * g + (0 if r == 0 else 8) + reg_runs[g][r]
        reg_runs[g][r] += 1
        upart[ui] = base
        ugroup[ui] = g
        gmain[g] += int(main_cnt[ui])
    # sc units into slots {4..7, 12..15}
    sc_runs = [[0, 0] for _ in range(8)]
    scs = sorted(SC_UNITS, key=lambda ui: -cls_cnt[ui])
    for ui in scs:
        best, bestv = -1, None
        for g in range(8):
            if sc_runs[g][0] >= 4 and sc_runs[g][1] >= 4:
                continue
            if bestv is None or gcls[g] < bestv:
                best, bestv = g, gcls[g]
        assert best >= 0, "sc packing overflow"
        g = best
        r = 0 if sc_runs[g][0] < 4 else 1
        base = 16 * g + (4 if r == 0 else 12) + sc_runs[g][r]
        sc_runs[g][r] += 1
        upart[ui] = base
        ugroup[ui] = g
        gcls[g] += int(cls_cnt[ui])
    return upart, ugroup, gmain, gcls


def _first_mask(cids):
    """Boolean mask selecting one 'first' entry per distinct cell id."""
    order = np.argsort(cids, kind="stable")
    sc = cids[order]
    fs = np.ones(len(cids), np.bool_)
    if len(cids):
        fs[1:] = sc[1:] != sc[:-1]
    mask = np.zeros(len(cids), np.bool_)
    mask[order] = fs
    return mask


def _build_bass(NVS, C0S, WB, NCLS):
    nc = bacc.Bacc("TRN2", target_bir_lowering=False)
    NIS = NVS // 16
    MEGA = nc.dram_tensor("mega", [128, WB], mybir.dt.uint8, kind="ExternalInput")
    OUT = nc.dram_tensor("out", [128, 8], mybir.dt.float32, kind="ExternalOutput")

    o_idx = 3072
    o_tms = o_idx + 2 * NIS
    o_mc = o_tms + 2 * NVS
    o_T = WB - 3072                 # phase A = [0, o_T)

    f32 = mybir.dt.float32
    bf16 = mybir.dt.bfloat16
    u16 = mybir.dt.uint16
    with tile.TileContext(nc) as tc:
        with tc.tile_pool(name="p", bufs=1) as pool:
            mega = pool.tile([128, WB], mybir.dt.uint8)
            # phase A: data + gather idx + small targets + cls mask
            nc.sync.dma_start(mega[:, 0:o_T], MEGA[:, 0:o_T])
            # phase B: dense targets, only the vp/sd rows (group-relative
            # slots {0..3} and {8..11}); score-row T stays garbage and the
            # host never reads those partitions of the dense sums
            nc.sync.dma_start(mega[:, o_T:WB], MEGA[:, o_T:WB])

            # warm activations: prefetch both tables off the critical path
            warm = pool.tile([128, 4], f32)
            nc.scalar.activation(warm[:, 0:2], warm[:, 2:4],
                                 mybir.ActivationFunctionType.Ln)
            nc.scalar.activation(warm[:, 0:2], warm[:, 2:4],
                                 mybir.ActivationFunctionType.Exp)
            nc.scalar.activation(warm[:, 0:2], warm[:, 2:4],
                                 mybir.ActivationFunctionType.Square)
            bm1 = pool.tile([128, 1], f32)
            nc.vector.memset(bm1[:], -1.0)

            data_v = mega[:, 0:3072].bitcast(bf16)               # [128,1536]
            idx_v = mega[:, o_idx:o_idx + 2 * NIS].bitcast(u16)
            tms_v = mega[:, o_tms:o_tms + 2 * NVS].bitcast(bf16)  # [128,NVS]
            mc_v = mega[:, o_mc:o_mc + NCLS]                     # u8
            T_v = mega[:, o_T:WB].bitcast(bf16)                  # [128,1536]

            P = pool.tile([128, 8], f32)

            # ---- small gather: dup-cell entries + cls logit pairs --------
            gs = pool.tile([128, NVS], bf16)
            nc.gpsimd.indirect_copy(
                gs[:], data_v, idx_v, i_know_ap_gather_is_preferred=True
            )

            # ---- small-chain smooth-l1 (dup entries; junk cols give 0) ---
            ds = pool.tile([128, NVS], bf16)
            nc.vector.tensor_tensor(ds[:], gs[:], tms_v,
                                    op=mybir.AluOpType.subtract)
            ds2 = pool.tile([128, NVS], bf16)
            nc.vector.tensor_tensor(ds2[:], ds[:], ds[:],
                                    op=mybir.AluOpType.mult)
            nc.vector.tensor_reduce(P[:, 2:3], ds2[:],
                                    axis=mybir.AxisListType.X,
                                    op=mybir.AluOpType.add)
            as_ = pool.tile([128, NVS], bf16)
            nc.vector.tensor_scalar(as_[:].bitcast(u16), ds[:].bitcast(u16),
                                    0x7FFF, None, mybir.AluOpType.bitwise_and)
            zs = pool.tile([128, NVS], bf16)
            nc.vector.tensor_scalar(zs[:], as_[:], 1.0, None,
                                    mybir.AluOpType.max)
            # cls difference early so the scalar engine can run Softplus
            # while the dense chain still occupies the DVE
            dc = pool.tile([128, NCLS], f32)
            nc.vector.tensor_tensor(dc[:], gs[:, C0S:NVS:2],
                                    gs[:, C0S + 1:NVS:2],
                                    op=mybir.AluOpType.subtract)

            sq2_s = pool.tile([128, NVS], bf16)
            nc.scalar.activation(sq2_s[:], zs[:],
                                 mybir.ActivationFunctionType.Square,
                                 bias=bm1[:], accum_out=P[:, 3:4])
            # ce = softplus(dc) = ln(exp(dc) + 1); the ln's table reload is
            # auto-inserted HERE, early, overlapping the dense DVE ops
            ex = pool.tile([128, NCLS], f32)
            nc.scalar.activation(ex[:], dc[:],
                                 mybir.ActivationFunctionType.Exp)
            ce = pool.tile([128, NCLS], f32)
            nc.scalar.activation(ce[:], ex[:],
                                 mybir.ActivationFunctionType.Ln, bias=1.0)

            # ---- dense smooth-l1 pass ------------------------------------
            D = pool.tile([128, 1536], bf16)
            nc.vector.tensor_tensor(D[:], data_v, T_v,
                                    op=mybir.AluOpType.subtract)
            A = pool.tile([128, 1536], bf16)
            nc.vector.tensor_scalar(A[:].bitcast(u16), D[:].bitcast(u16),
                                    0x7FFF, None, mybir.AluOpType.bitwise_and)
            Z = pool.tile([128, 1536], bf16)
            nc.vector.tensor_scalar(Z[:], A[:], 1.0, None,
                                    mybir.AluOpType.max)
            SQ2 = pool.tile([128, 1536], bf16)
            nc.scalar.activation(SQ2[:], Z[:],
                                 mybir.ActivationFunctionType.Square,
                                 bias=bm1[:], accum_out=P[:, 1:2])
            D2 = pool.tile([128, 1536], bf16)
            nc.vector.tensor_tensor(D2[:], D[:], D[:],
                                    op=mybir.AluOpType.mult)
            nc.vector.tensor_reduce(P[:, 0:1], D2[:],
                                    axis=mybir.AxisListType.X,
                                    op=mybir.AluOpType.add)

            # cls mask-and-sum
            cj = pool.tile([128, NCLS], f32)
            nc.vector.tensor_tensor(cj[:], ce[:], mc_v,
                                    op=mybir.AluOpType.mult)
            nc.vector.tensor_reduce(P[:, 4:5], cj[:],
                                    axis=mybir.AxisListType.X,
                                    op=mybir.AluOpType.add)
            nc.vector.memset(P[:, 5:8], 0.0)

            nc.sync.dma_start(OUT[:, :], P[:])
    nc.compile()
    return nc


def kernel(**inputs):
    score = np.asarray(inputs["score"], dtype=np.float32)[0]            # [20,H,W]
    vp = np.asarray(inputs["vertical_pred"], dtype=np.float32)[0]
    side = np.asarray(inputs["side_refinement"], dtype=np.float32)[0]   # [10,H,W]
    pidx = np.asarray(inputs["positive"])
    nidx = np.asarray(inputs["negative"])
    vidx = np.asarray(inputs["vertical_reg_idx"])
    vtgt = np.asarray(inputs["vertical_reg_tgt"], dtype=np.float32)
    sidx = np.asarray(inputs["side_reg_idx"])
    stgt = np.asarray(inputs["side_reg_tgt"], dtype=np.float32)

    score_bf = _bf16(score.reshape(2 * K, HW))
    vp_bf = _bf16(vp.reshape(2 * K, HW))
    side_bf = _bf16(side.reshape(K, HW))
    vtgt_bf = _bf16(vtgt)      # [Nv,2] u16
    stgt_bf = _bf16(stgt)      # [No]   u16

    def fields(idx):
        x = idx[:, 0].astype(np.int64)
        y = idx[:, 1].astype(np.int64)
        a = idx[:, 2].astype(np.int64)
        pos = y * W + x
        return a, pos // PPC, pos % PPC

    va, vcore, vposl = fields(vidx)
    sa, score_, sposl = fields(sidx)
    pa, pcore, pposl = fields(pidx)
    na, ncore, nposl = fields(nidx)

    # --- per-entry unit / in-row-offset ----------------------------------
    v_h = vposl // COLS
    v_u = (vposl % COLS).astype(np.int64)
    v_unit = (va * 2 + v_h).astype(np.int64)                 # vp units 0..19
    s_h = sposl // COLS
    s_u = (sposl % COLS).astype(np.int64)
    s_unit = (20 + sa * 2 + s_h).astype(np.int64)            # sd units 20..39
    p_q = pposl // QCOLS
    p_u = (2 * (pposl % QCOLS)).astype(np.int64)
    p_unit = (40 + pa * 4 + p_q).astype(np.int64)            # sc units 40..79
    n_q = nposl // QCOLS
    n_u = (2 * (nposl % QCOLS)).astype(np.int64)
    n_unit = (40 + na * 4 + n_q).astype(np.int64)

    # --- dup split: one dense anchor per distinct cell --------------------
    v_cid = (vcore * K + va) * PPC + vposl                   # vp pair-cell id
    s_cid = K * HW + (score_ * K + sa) * PPC + sposl         # sd cell id
    v_first = _first_mask(v_cid)
    s_first = _first_mask(s_cid)

    main_core = np.concatenate([vcore, score_])
    main_unit = np.concatenate([v_unit, s_unit])
    main_u = np.concatenate([v_u, s_u])
    main_t0 = np.concatenate([vtgt_bf[:, 0], stgt_bf])       # bf16 bits
    main_t1 = np.concatenate([vtgt_bf[:, 1],
                              np.zeros_like(stgt_bf)])
    main_isv = np.concatenate(
        [np.ones(len(vidx), np.bool_), np.zeros(len(sidx), np.bool_)])
    main_first = np.concatenate([v_first, s_first])

    cls_core = np.concatenate([pcore, ncore])
    cls_unit = np.concatenate([p_unit, n_unit])
    cls_u = np.concatenate([p_u, n_u])
    cls_ispos = np.concatenate(
        [np.ones(len(pidx), np.bool_), np.zeros(len(nidx), np.bool_)])

    dup_cnt = np.zeros((N_CORES, N_UNITS), np.int64)
    dsel_all = ~main_first
    np.add.at(dup_cnt, (main_core[dsel_all], main_unit[dsel_all]), 1)
    cls_cnt = np.zeros((N_CORES, N_UNITS), np.int64)
    np.add.at(cls_cnt, (cls_core, cls_unit), 2)

    packs = [_pack_units(dup_cnt[c], cls_cnt[c]) for c in range(N_CORES)]
    c0s = max(max(p[2]) for p in packs)
    c0s += c0s % 2
    max_cls = max(max(p[3]) for p in packs)
    NVS = c0s + max_cls
    NVS = max(16, ((NVS + 15) // 16) * 16)
    NCLS = (NVS - c0s) // 2
    NIS = NVS // 16
    WB = 3072 + 2 * NIS + 2 * NVS + NCLS
    WB = ((WB + 3) // 4) * 4
    WB += 3072                       # trailing dense-target region

    key = (NVS, c0s)
    if key not in _cache:
        _cache[key] = _build_bass(NVS, c0s, WB, NCLS)
    nc = _cache[key]

    o_idx = 3072
    o_tms = o_idx + 2 * NIS
    o_mc = o_tms + 2 * NVS
    o_T = WB - 3072

    in_maps = []
    wvec_v = np.zeros((N_CORES, 128), np.float32)
    wvec_o = np.zeros((N_CORES, 128), np.float32)
    for c in range(N_CORES):
        upart, ugroup, gmain, gcls = packs[c]
        uparta = np.asarray(upart, np.int64)

        # dense data tile (bf16 bits) --------------------------------------
        df = np.zeros((128, COLS), np.uint16)
        base = c * PPC
        for ui, (kind, a, hq) in enumerate(UNITS):
            p0 = upart[ui]
            if kind == "vp":
                sl = slice(base + hq * COLS, base + (hq + 1) * COLS)
                df[p0] = vp_bf[2 * a, sl]
                df[p0 + 1] = vp_bf[2 * a + 1, sl]
                wvec_v[c, p0] = wvec_v[c, p0 + 1] = 1.0 / (2.0 * NV_REG)
            elif kind == "sd":
                sl = slice(base + hq * COLS, base + (hq + 1) * COLS)
                df[p0] = side_bf[a, sl]
                wvec_o[c, p0] = 1.0 / NO_REG
            else:  # sc, pair-interleaved quarter
                sl = slice(base + hq * QCOLS, base + (hq + 1) * QCOLS)
                df[p0, 0::2] = score_bf[2 * a, sl]
                df[p0, 1::2] = score_bf[2 * a + 1, sl]

        # dense target tile: data copy, then first-occurrence targets ------
        Tf = df.copy()
        msel = (main_core == c) & main_first
        Tf[uparta[main_unit[msel]], main_u[msel]] = main_t0[msel]
        vsel = msel & main_isv
        Tf[uparta[main_unit[vsel]] + 1, main_u[vsel]] = main_t1[vsel]

        # small gather: dup entries + cls pairs ----------------------------
        idxs = np.zeros((128, NIS), np.uint16)
        ucol = np.zeros((8, NVS), np.int64)
        mcls = np.zeros((128, NCLS), np.uint8)

        gq_main = [0] * 8
        gq_cls = [0] * 8

        def put_idx(g, col, val):
            idxs[16 * g + col % 16, col // 16] = val
            ucol[g, col] = val

        ov_p, ov_c, ov_t = [], [], []
        dsel = (main_core == c) & ~main_first
        for u, ui, t0, t1, isv in zip(main_u[dsel], main_unit[dsel],
                                      main_t0[dsel], main_t1[dsel],
                                      main_isv[dsel]):
            g = ugroup[ui]
            col = gq_main[g]
            gq_main[g] += 1
            put_idx(g, col, u)
            p0 = upart[ui]
            ov_p.append(p0); ov_c.append(col); ov_t.append(t0)
            if isv:
                ov_p.append(p0 + 1); ov_c.append(col); ov_t.append(t1)

        csel = cls_core == c
        for u, ui, ispos in zip(cls_u[csel], cls_unit[csel],
                                cls_ispos[csel]):
            g = ugroup[ui]
            i = gq_cls[g]
            gq_cls[g] += 1
            colf = c0s + 2 * i
            if ispos:
                put_idx(g, colf, u)
                put_idx(g, colf + 1, u + 1)
            else:
                put_idx(g, colf, u + 1)
                put_idx(g, colf + 1, u)
            mcls[upart[ui], i] = 1

        # small TM (single plane, bf16): default = the gathered bits, so
        # junk columns (incl. the whole cls region) subtract to exactly 0
        tms = np.empty((128, NVS), np.uint16)
        for g in range(8):
            tms[16 * g:16 * g + 16] = df[16 * g:16 * g + 16][:, ucol[g]]
        if ov_p:
            tms[np.array(ov_p), np.array(ov_c)] = np.array(ov_t, np.uint16)

        mega = np.zeros((128, WB), np.uint8)
        mega[:, 0:3072] = df.view(np.uint8)
        mega[:, o_idx:o_idx + 2 * NIS] = idxs.view(np.uint8)
        mega[:, o_tms:o_tms + 2 * NVS] = tms.view(np.uint8)
        mega[:, o_mc:o_mc + NCLS] = mcls
        mega[:, o_T:WB] = Tf.view(np.uint8)
        in_maps.append({"mega": mega})

    res = bass_utils.run_bass_kernel_spmd(
        nc, in_maps, core_ids=list(range(N_CORES)))

    v_loss = np.float32(0.0)
    o_loss = np.float32(0.0)
    cls_sum = np.float32(0.0)
    for c in range(N_CORES):
        P = res.results[c]["out"]      # [128, 8]
        S = 0.5 * (P[:, 0] - P[:, 1]) + 0.5 * (P[:, 2] - P[:, 3])
        # score-row partitions carry garbage dense sums (no T shipped
        # there); select them away before the weighted dot
        m = (wvec_v[c] != 0) | (wvec_o[c] != 0)
        S = np.where(m, S, np.float32(0))
        v_loss += np.float32(np.dot(S, wvec_v[c]))
        o_loss += np.float32(np.dot(S, wvec_o[c]))
        cls_sum += np.float32(P[:, 4].sum())
    cls_loss = np.float32(cls_sum / NS)
    loss = np.float32(cls_loss + v_loss + o_loss)
    return (np.float32(loss), np.float32(cls_loss), np.float32(v_loss),
            np.float32(o_loss))
